# revision 11
# baseline (speedup 1.0000x reference)
"""BoxBlur 7x7 (normalized, reflect padding) on 8 Trainium2 NeuronCores.

Strategy (pure data parallel, 4 images x 3 channels = 12 image-planes per core).
The workload is chip-HBM-bandwidth bound (~100.7 MB/core irreducible traffic,
pure-DMA floor ~255 us on 8 concurrent cores), so the kernel is organized so
no compute engine ever gates the two DMA streams:

  - Host-side padded rows: each DRAM row is [7 zeros | r3 r2 r1 | x0..x1023 |
    r1' r2' r3'] (1040 f32 cols, 4160 B descriptors).  The 7-zero prefix makes
    the horizontal scan's initial state exactly 0 and the reflect columns come
    in with the data, so the on-chip horizontal pass is ONE DVE op per tile
    (no memset / reflect copies / tensor_reduce).  Load once: the host also
    prepends 3 zero rows, putting the h-tile grid at x rows 128t-3..128t+124.
  - Loads ride the sync HWDGE ring, stores the scalar HWDGE ring (two
    physical HW-DGE rings; keeping them separate avoids FIFO coupling and
    leaves GpSimd/Q7 completely idle).
  - Horizontal 7-tap box sum per 128-row tile on VectorE: one rolling scan
    (tensor_tensor_scan, fp32 in, bf16 out) h[c] = h[c-1] + q[c+7] - q[c]
    over the padded row; the window sum of the 7-zero prefix region is the
    init, i.e. 0.  Valid outputs are h[:, 6:1030].
  - Vertical 7-tap weighted sum on TensorE in bf16: out tile u [128, 1024]
    accumulates two banded matmuls per 512-column PSUM bank:
    W_a[128,128].T @ h_u + W_b[6,128].T @ h_{u+1}[0:6].  The bf16 band
    matrices carry the 1/49 normalization and the vertical reflect folding
    at each image-plane top/bottom (3 tile kinds: top/interior/bottom).
  - PSUM evacuated to an SBUF staging buffer on ScalarE (fp32); stores go
    out as batched 1 MB transfers on the scalar HWDGE ring.
"""

import numpy as np

import concourse.bass as bass
import concourse.tile as tile
from concourse import bacc, mybir
from concourse.bass_utils import run_bass_kernel_spmd

H = W = 1024
KH = KW = 7
PAD = 3              # k // 2
SEG = 128            # h-tile / out-tile row count
N_CORES = 8
IMGS_PER_CORE = 4    # 32 / 8
CHANNELS = 3
IC_PER_CORE = IMGS_PER_CORE * CHANNELS       # 12 image-planes per core
ROWS = IC_PER_CORE * H                       # 12288
XROWS = ROWS + PAD                           # 12291 (3 zero rows on top)
N_TILES = ROWS // SEG                        # 96 h tiles (+1 runt) = out tiles
TPP = H // SEG                               # 8 tiles per plane

# padded DRAM row: [7 zeros][x3 x2 x1][x0..x1023][x1022 x1021 x1020][3 slack]
ZPRE = KW           # 7-col zero prefix (scan warm-up)
XOFF = ZPRE + PAD   # 10: col of x0
QCOLS = ZPRE + PAD + W + PAD     # 1037 meaningful columns
XCOLS = 1040        # row allocation (16B aligned)
HVALID = 6          # h[:, 6:6+W] are the valid horizontal sums

_F32 = mybir.dt.float32
_BF16 = mybir.dt.bfloat16

LOAD_BATCH = 1                     # h tiles per load dma (~0.53 MB)
STORE_BATCH = 1                    # out tiles per store dma (0.5 MB)
XBUFS = 28
HBUFS = 10
OBUFS = 6
PSUM_BUFS = 4

_compiled = None  # cached compiled Bass program


def _build_weights(kcol):
    """Band matrices for the vertical pass.

    h_tile t partition j corresponds to x row 128t + j - PAD (plane-local
    row (128t + j - PAD) % H).  Out tile u row m = x row 128u + m.  Returns
    wa_top/wa_int/wa_bot [128,128] and wb_int [6,128] / wb_bot [3,128]
    (wb applies to h_{u+1}; plane-bottom tiles only touch its first 3 rows).
    """
    def build(pos):
        Wa = np.zeros((SEG, SEG), np.float32)
        Wb = np.zeros((KW - 1, SEG), np.float32)
        for m in range(SEG):
            r_loc = pos * SEG + m            # plane-local out row
            for d in range(-PAD, PAD + 1):
                r = r_loc + d
                if r < 0:
                    r = -r
                if r > H - 1:
                    r = 2 * (H - 1) - r
                j = (r - pos * SEG) + PAD    # partition in h grid space
                if j < SEG:
                    Wa[j, m] += kcol[d + PAD]
                else:
                    Wb[j - SEG, m] += kcol[d + PAD]
        return Wa, Wb

    wa_top, wb_top = build(0)
    wa_int, wb_int = build(3)
    wa_bot, wb_bot = build(TPP - 1)
    assert np.array_equal(wb_top, wb_int)
    assert not wb_bot[PAD:].any()
    return wa_top, wa_int, wa_bot, wb_int, wb_bot[:PAD]


def _body(tc, nc, x, ws, out):
    with (
        tc.tile_pool(name="wpool", bufs=1) as wpool,
        tc.tile_pool(name="xpad", bufs=XBUFS) as xpool,
        tc.tile_pool(name="hbuf", bufs=HBUFS) as hpool,
        tc.tile_pool(name="psum", bufs=PSUM_BUFS, space="PSUM") as ppool,
        tc.tile_pool(name="osb", bufs=OBUFS) as opool,
    ):
        # --- weights (scalar ring; sync ring starts on x immediately) ---
        wa_t = []
        for i, name in enumerate(("wa_top", "wa_int", "wa_bot")):
            t = wpool.tile([128, SEG], _BF16, tag=name)
            nc.scalar.dma_start(t[:, :], ws[name])
            wa_t.append(t)
        wb_int_t = wpool.tile([128, SEG], _BF16, tag="wb_int")
        nc.scalar.dma_start(wb_int_t[0 : KW - 1, :], ws["wb_int"])
        wb_bot_t = wpool.tile([128, SEG], _BF16, tag="wb_bot")
        nc.scalar.dma_start(wb_bot_t[0:PAD, :], ws["wb_bot"])
        zinit = wpool.tile([128, 8], _F32, tag="zinit")
        nc.vector.memset(zinit[:, :], 0.0)

        def hscan(hview, xview, P, eng=None):
            (eng or nc.vector).tensor_tensor_scan(
                hview[0:P, 0 : QCOLS - ZPRE],
                xview[0:P, ZPRE:QCOLS],
                xview[0:P, 0 : QCOLS - ZPRE],
                zinit[0:P, 0:1],
                op0=mybir.AluOpType.add,
                op1=mybir.AluOpType.subtract,
            )

        # --- runt h tile (last 3 rows of the core block), computed once.
        # Loaded on the scalar ring (sync ring starts batch loads at once);
        # its scan is emitted mid-loop (DVE bubble) since the result is only
        # needed for the very last out tile.
        xr = xpool.tile([128, XCOLS], _F32, tag="xr", bufs=1)
        nc.scalar.dma_start(xr[0:PAD, 0:XCOLS], x[N_TILES * SEG : XROWS, :])
        h_runt = hpool.tile([128, 1032], _BF16, tag="hrunt", bufs=1)

        n_batches = N_TILES // LOAD_BATCH
        h_tiles = {N_TILES: h_runt}
        obuf = None

        def emit_out(u):
            nonlocal obuf
            pos = u % TPP
            kind = 0 if pos == 0 else (2 if pos == TPP - 1 else 1)
            wa = wa_t[kind]
            wb, k2 = (wb_bot_t, PAD) if kind == 2 else (wb_int_t, KW - 1)
            h_u = h_tiles[u]
            h_n = h_tiles[u + 1]
            ps = ppool.tile([128, W], _F32, tag="ps")
            # fp32 PSUM output maxes at 512 columns (one bank) per matmul.
            for half in range(2):
                sl = slice(HVALID + half * 512, HVALID + (half + 1) * 512)
                osl = slice(half * 512, (half + 1) * 512)
                nc.tensor.matmul(
                    ps[:, osl], wa[:, :], h_u[:, sl], start=True, stop=False
                )
                nc.tensor.matmul(
                    ps[:, osl], wb[0:k2, :], h_n[0:k2, sl], start=False, stop=True
                )
            if u % STORE_BATCH == 0:
                obuf = opool.tile([128, STORE_BATCH * W], _F32, tag="ob")
            nc.scalar.copy(
                obuf[:, (u % STORE_BATCH) * W : (u % STORE_BATCH + 1) * W],
                ps[:, :],
            )
            if u % STORE_BATCH == STORE_BATCH - 1:
                sb = u // STORE_BATCH
                rows = STORE_BATCH * SEG
                dview = out[sb * rows : (sb + 1) * rows, :].rearrange(
                    "(s p) w -> p s w", s=STORE_BATCH
                )
                sview = obuf[:, :].rearrange("p (s w) -> p s w", s=STORE_BATCH)
                nc.scalar.dma_start(dview, sview)
            del h_tiles[u]

        for bt in range(n_batches):
            xb = xpool.tile([128, LOAD_BATCH * XCOLS], _F32, tag="xb")
            rows = LOAD_BATCH * SEG
            dview = x[bt * rows : (bt + 1) * rows, :].rearrange(
                "(s p) w -> p s w", s=LOAD_BATCH
            )
            sview = xb[:, :].rearrange("p (s b) -> p s b", s=LOAD_BATCH)
            nc.sync.dma_start(sview, dview)
            for j in range(LOAD_BATCH):
                t = bt * LOAD_BATCH + j
                xv = xb[:, j * XCOLS : (j + 1) * XCOLS]
                h = hpool.tile([128, 1032], _BF16, tag="h")
                hscan(h, xv, 128)
                h_tiles[t] = h
                if t == N_TILES // 2:
                    hscan(h_runt, xr, PAD)
                if t >= 1:
                    emit_out(t - 1)
        emit_out(N_TILES - 1)


def _bass_program(num_devices=N_CORES):
    nc = bacc.Bacc(
        "TRN2",
        target_bir_lowering=False,
        debug=False,
        enable_asserts=False,
        num_devices=num_devices,
    )
    x_ap = nc.dram_tensor("x", [XROWS, XCOLS], _F32, kind="ExternalInput").ap()
    ws = {}
    for name, shape in (
        ("wa_top", [SEG, SEG]),
        ("wa_int", [SEG, SEG]),
        ("wa_bot", [SEG, SEG]),
        ("wb_int", [KW - 1, SEG]),
        ("wb_bot", [PAD, SEG]),
    ):
        ws[name] = nc.dram_tensor(name, shape, _BF16, kind="ExternalInput").ap()
    out_ap = nc.dram_tensor("out", [ROWS, W], _F32, kind="ExternalOutput").ap()
    with tile.TileContext(nc) as tc:
        _body(tc, nc, x_ap, ws, out_ap)
    nc.compile()
    return nc


def _get_program():
    global _compiled
    if _compiled is None:
        _compiled = _bass_program()
    return _compiled


def _make_in_maps(x, kernel):
    import ml_dtypes

    x = np.ascontiguousarray(np.asarray(x, dtype=np.float32))
    assert x.shape == (N_CORES * IMGS_PER_CORE, CHANNELS, H, W), x.shape
    k2 = np.asarray(kernel, dtype=np.float64)
    k2 = k2 / k2.sum()
    # horizontal pass is an unweighted 7-tap sum => all columns of the
    # normalized kernel must be identical (true for the box kernel).
    assert np.allclose(k2, k2[:, :1]), "kernel must have uniform rows"
    kcol = k2[:, 0].astype(np.float32)
    wa_top, wa_int, wa_bot, wb_int, wb_bot = (
        w.astype(ml_dtypes.bfloat16)
        for w in _build_weights(kcol)
    )
    xr = x.reshape(N_CORES, ROWS, W)
    xp = np.zeros((N_CORES, XROWS, XCOLS), np.float32)
    xp[:, PAD:, XOFF : XOFF + W] = xr
    # left reflect cols 7,8,9 = x[3],x[2],x[1]
    xp[:, PAD:, ZPRE : ZPRE + PAD] = xr[:, :, PAD:0:-1]
    # right reflect cols 1034,1035,1036 = x[1022],x[1021],x[1020]
    xp[:, PAD:, XOFF + W : QCOLS] = xr[:, :, W - 2 : W - PAD - 2 : -1]
    return [
        {
            "x": xp[c],
            "wa_top": wa_top,
            "wa_int": wa_int,
            "wa_bot": wa_bot,
            "wb_int": wb_int,
            "wb_bot": wb_bot,
        }
        for c in range(N_CORES)
    ]


def run_shards(in_maps, **kwargs):
    """Compile (cached) + run on cores 0..7; returns BassKernelResults."""
    nc = _get_program()
    return run_bass_kernel_spmd(nc, in_maps, core_ids=list(range(N_CORES)), **kwargs)


def kernel(x, kernel):
    in_maps = _make_in_maps(x, kernel)
    try:
        res = run_shards(in_maps)
    except Exception:
        # one retry: transient NRT device errors have been observed under
        # the PJRT/axon path; the device recovers on a fresh dispatch.
        import time as _time

        _time.sleep(30)
        res = run_shards(in_maps)
    outs = [
        res.results[c]["out"].reshape(IMGS_PER_CORE, CHANNELS, H, W)
        for c in range(N_CORES)
    ]
    return np.concatenate(outs, axis=0)
